# revision 36
# baseline (speedup 1.0000x reference)
"""KAN expert kernel for Trainium2 (8 NeuronCores, data-parallel over batch).

Math: out[b,j] = sum_{i,g} basis_g(x[b,i]) * coeff[i,j,g] * scaling[i,j]
with cubic B-spline basis on the uniform extended grid g_m = -1 + 0.4*m.

Key identity (truncated powers): for the uniform grid, the basis is the
cardinal cubic B-spline, basis_g(x) = (1/(6h^3)) * sum_{r=0..4} w_r *
relu(x - g_{g+r})^3 with w = [1,-4,6,-4,1]. Since x in [-1,1) only
relu-features m=0..4 are nonzero, and the (linear) binomial combine is
folded into the weights on the host:
    C'[m,i,j] = (1/(6h^3)) * sum_g w_{m-g} * coeff[i,j,g] * scaling[i,j]
so each core computes Q_m = relu(x - g_m)^3 (m=0..4) and a
[512b x 2560k] @ [2560k x 512j] fp16 matmul accumulated in fp32 PSUM.

Precision: the truncated-power split cancels heavily (sum |q*W| >>
|sum q*W|), so the matmul INPUTS need >= 10 mantissa bits for the
large-magnitude channels m0..m2: those compute in fp32 and round to
fp16 once:
    r_m = max(x - g_m, 0)     (DVE tensor_scalar, fp32)
    s_m = Square(x - g_m)     (ACT bias form, fp32; == r^2 wherever r>0)
    q_m = fp16(r_m * s_m)     (DVE tensor_mul, single rounding)
Channels m3/m4 have |q| <= 0.5 so a full fp16 chain is harmless
(host-emulated rel err 7.86e-3 vs 7.84e-3 all-fp32; the 2e-2 gate has
2.5x margin) and fp16 doubles the DVE rate for them.

Schedule notes (measured on HW):
 - the measured window runs from Bass's builtin const memsets (~6.3us
   into the NEFF) to the end of the framework's ~7.1us semaphore-clear
   teardown; the kernel body must minimize first-X-byte -> last-out-DMA.
 - the bias consts for the squares are registered with gpsimd memsets
   that then_inc a semaphore awaited once by the Scalar engine (instead
   of an all_engine_barrier, which would delay the X DMA issue ~0.9us).
 - X lands in two pieces: ic0 (sync queue) gating the first 512-wide
   feature chunk, and ic1..3 (scalar queue). W groups chain behind the
   X pieces on two queues so early tensors get full DMA bandwidth in
   consumption order.  The ACT squares take their bias from X directly,
   so ACT production is decoupled from DVE.
 - the PE consumes a 512-col Q chunk (4 matmuls) in 0.86us, exactly the
   DVE's fp32 production rate (R 0.55 + mul 1.11 ns/col), so the stream
   is just-in-time through m1/m2: the real matmuls must start no
   earlier than T* ~ 14us or the PE stalls AND declocks 2x (427ns/MM,
   ~2us to re-ramp).  The warmup queue over a dummy tile is sized to
   drain exactly at T*; it also ramps the PE clock from its idle state.
 - LDWEIGHTS+MATMUL pairs sustain ~216ns/MM when fed; each half carries
   at most one sync wait.  m0..m3 run ic-outer (chunk-gated), m4
   bc-outer so each psum's eviction + output DMA overlap the tail.
"""

import numpy as np

BATCH = 4096
IN_DIM = 512
OUT_DIM = 512
GRID_SIZE = 5
K = 3
N_CORES = 8
P = 128
NM = 5                      # relu^3 feature channels
BC = BATCH // N_CORES       # 512 batch rows per core
NIC = IN_DIM // P           # 4 input-dim chunks
NBC = BC // P               # 4 batch chunks (psum tiles)

_W_BINOM = np.array([1.0, -4.0, 6.0, -4.0, 1.0])
_FP16_MS = {3, 4}           # full-fp16 feature chains (see docstring)

_cached = {}


def _grid_f32():
    h = 2.0 / GRID_SIZE
    return np.float32(-1.0 + h * np.arange(GRID_SIZE + 2 * K + 1))


def _build_nc(mm_dtype_name="float16", warmup=16, warmup_short=8):
    import concourse.bass as bass
    import concourse.mybir as mybir
    from concourse.tile import TileContext
    from concourse.bass import _add_dep_helper

    dt = mybir.dt
    mm_dt = getattr(dt, mm_dtype_name)
    grid = _grid_f32()

    nc = bass.Bass()
    # bias constants for the Square/Relu activations.  The memsets run
    # on gpsimd right after Bass's builtin-const block; the Scalar
    # engine (the only consumer) waits once on their semaphore, so the
    # sync/scalar sequencers stay free to issue the input DMAs early.
    csem = nc.alloc_semaphore("cbias")
    _n = 0
    for _m in range(NM):
        _v = float(-grid[_m])
        if (dt.float32, _v) not in nc.const_aps.aps:
            _t = nc.alloc_sbuf_tensor(f"const-float32-{_v}", [128, 1],
                                      dt.float32)
            nc.gpsimd.memset(_t.ap(), _v).then_inc(csem, 1)
            nc.const_aps.aps[(dt.float32, _v)] = _t.ap()
            _n += 1
    nc.scalar.wait_ge(csem, _n)

    xt = nc.dram_tensor("xt", [IN_DIM, BC], mm_dt, kind="ExternalInput")
    cw = nc.dram_tensor("cw", [NM * IN_DIM, OUT_DIM], mm_dt,
                        kind="ExternalInput")
    out = nc.dram_tensor("out", [BC, OUT_DIM], mm_dt,
                         kind="ExternalOutput")

    ACTF = mybir.ActivationFunctionType
    ALU = mybir.AluOpType

    def ft_dt(m):
        return mm_dt if m in _FP16_MS else dt.float32

    # R/S/Q column chunks per m (ic-major feature space [128, 2048]):
    # m0 starts 512-wide (ic0 lands first, minimizes X->first-MM
    # latency); fp32 muls cap at 1024 (tensor_tensor degrades wider);
    # fp16 m3/m4 are cheap either way.
    RCH = {0: [(0, 512), (512, 1024), (1024, 2048)],
           1: [(0, 1024), (1024, 2048)],
           2: [(0, 2048)],
           3: [(0, 2048)],
           4: None}                      # m4 relu runs on ACT, full
    SCH = {0: [(0, 512), (512, 1024), (1024, 2048)],
           1: [(0, 1024), (1024, 2048)],
           2: [(0, 2048)],
           3: [(0, 2048)],
           4: [(0, 2048)]}
    QCH = {0: [(0, 512), (512, 1024), (1024, 2048)],
           # m1/m2 muls in 512-col chunks: each matmul quad gates on a
           # quarter-chunk, moving the just-in-time bound ~0.45us earlier
           1: [(0, 512), (512, 1024), (1024, 1536), (1536, 2048)],
           2: [(0, 512), (512, 1024), (1024, 1536), (1536, 2048)],
           3: [(0, 1024), (1024, 2048)],
           4: [(0, 1024), (1024, 2048)]}

    with TileContext(nc) as tc:
        with tc.tile_pool(name="main", bufs=1) as pool, \
             tc.tile_pool(name="psum", bufs=1, space="PSUM") as psum_pool:
            X = pool.tile([P, NIC * BC], mm_dt, tag="X")
            CW = pool.tile([P, NM * NIC * OUT_DIM], mm_dt, tag="CW")

            # PE warmup over a zeroed dummy tile: ramps the PE clock and
            # drains exactly at T* so the real stream never stalls.
            dumb = pool.tile([P, OUT_DIM], mm_dt, tag="dumb")
            dpsum = psum_pool.tile([P, OUT_DIM], dt.float32, tag="dps",
                                   name="dps")
            nc.gpsimd.memset(dumb[:], 0.0)
            for _ in range(warmup):
                nc.tensor.matmul(dpsum[:], dumb[:, 0:P], dumb[:],
                                 start=True, stop=True)
            for _ in range(warmup_short):
                nc.tensor.matmul(dpsum[:, 0:P], dumb[:, 0:P],
                                 dumb[:, 0:P], start=True, stop=True)

            # ---- input DMAs (partition-major k = p*NIC+t on both
            # matmul operands: 128 contiguous descriptors per group).
            xt_r = xt.rearrange("(p t) b -> p t b", p=P)

            def dma_x(eng, t0, t1):
                return getattr(nc, eng).dma_start(
                    out=X[:, t0 * BC:t1 * BC]
                        .rearrange("p (t b) -> p t b", t=t1 - t0),
                    in_=xt_r[:, t0:t1, :])

            def dma_w(m, t0, t1, eng="sync"):
                grp = cw[m * IN_DIM:(m + 1) * IN_DIM, :] \
                    .rearrange("(p t) j -> p t j", p=P)
                return getattr(nc, eng).dma_start(
                    out=CW[:, (m * NIC + t0) * OUT_DIM:
                           (m * NIC + t1) * OUT_DIM]
                        .rearrange("p (t j) -> p t j", t=t1 - t0),
                    in_=grp[:, t0:t1, :])

            xp0 = dma_x("sync", 0, 1)      # ic0: gates the first chunk
            xp1 = dma_x("scalar", 1, NIC)  # ic1..3
            dma_w(0, 0, 1, eng="gpsimd")   # W(m0, t0): first matmuls
            # two W chains staggered behind the X pieces so the early
            # tensors get the full DMA bandwidth in consumption order
            chain_a, chain_b = xp0, xp1
            for i, (m, t0, t1) in enumerate([(0, 1, NIC)]
                                            + [(m, 0, NIC)
                                               for m in range(1, NM)]):
                eng = "gpsimd" if m == NM - 1 else "sync"
                wd = dma_w(m, t0, t1, eng=eng)
                prev = chain_a if i % 2 == 0 else chain_b
                _add_dep_helper(wd.ins, prev.ins, sync=True,
                                reason="stagger W DMAs behind X/previous")
                if i % 2 == 0:
                    chain_a = wd
                else:
                    chain_b = wd

            def w_tile(m, ic):
                o = (m * NIC + ic) * OUT_DIM
                return CW[:, o:o + OUT_DIM]

            # ---- features
            R = [pool.tile([P, NIC * BC], ft_dt(m), tag=f"r{m}",
                           name=f"r{m}") for m in range(NM)]
            S = [pool.tile([P, NIC * BC], ft_dt(m), tag=f"s{m}",
                           name=f"s{m}") for m in range(NM)]
            Q = [pool.tile([P, NIC * BC], mm_dt, tag=f"q{m}",
                           name=f"q{m}") for m in range(NM)]

            prev_dve = [None]

            def dve_order(inst):
                if prev_dve[0] is not None:
                    _add_dep_helper(inst.ins, prev_dve[0].ins, sync=False,
                                    reason="DVE consumption order")
                prev_dve[0] = inst
                return inst

            def emit_R(m, lo, hi):
                gm = float(grid[m])
                for (c0, c1) in RCH[m]:
                    if lo <= c0 and c1 <= hi:
                        dve_order(nc.vector.tensor_scalar(
                            R[m][:, c0:c1], X[:, c0:c1], gm, 0.0,
                            ALU.subtract, ALU.max))

            def emit_S(m, lo, hi):
                gm = float(grid[m])
                for (c0, c1) in SCH[m]:
                    if lo <= c0 and c1 <= hi:
                        nc.scalar.activation(S[m][:, c0:c1], X[:, c0:c1],
                                             ACTF.Square, bias=-gm)

            def emit_Q(m, lo, hi):
                for (c0, c1) in QCH[m]:
                    if lo <= c0 and c1 <= hi:
                        dve_order(nc.vector.tensor_mul(
                            Q[m][:, c0:c1], R[m][:, c0:c1],
                            S[m][:, c0:c1]))

            W_TOT = NIC * BC
            # m0: R0a + its square + Q0a first (T0 gate), the rest of
            # m0 after (DMA-gated anyway)
            emit_R(0, 0, 512)
            emit_S(0, 0, 512)
            emit_Q(0, 0, 512)
            emit_R(0, 512, W_TOT)
            emit_S(0, 512, W_TOT)
            emit_Q(0, 512, W_TOT)
            # m1..m3: R, S (from X, decoupled), Q
            for m in (1, 2, 3):
                emit_R(m, 0, W_TOT)
                emit_S(m, 0, W_TOT)
                emit_Q(m, 0, W_TOT)
            # m4: the relu comes from ACT too, so the q-mult's two
            # producers share one Activation wait
            nc.scalar.activation(R[4][:], X[:], ACTF.Relu,
                                 bias=-float(grid[4]))
            emit_S(4, 0, W_TOT)
            emit_Q(4, 0, W_TOT)

            # ---- matmuls.  m0..m3 ic-outer (chunk-gated); m4 bc-outer
            # so each psum finishes early in the round and its eviction
            # + output DMA overlap the remaining matmuls.
            psums = [psum_pool.tile([P, OUT_DIM], dt.float32, tag=f"ps{b}",
                                    name=f"ps{b}")
                     for b in range(NBC)]
            O = pool.tile([P, NBC * OUT_DIM], mm_dt, tag="O")
            out_dmas = []

            def mm(m, bc, ic):
                kc = m * NIC + ic
                lhsT = Q[m][:, ic * BC + bc * P: ic * BC + (bc + 1) * P]
                nc.tensor.matmul(psums[bc][:], lhsT, w_tile(m, ic),
                                 start=(kc == 0),
                                 stop=(kc == NM * NIC - 1))

            for m in range(NM - 1):
                for ic in range(NIC):
                    for bc in range(NBC):
                        mm(m, bc, ic)
            for bc in range(NBC):
                for ic in range(NIC):
                    mm(NM - 1, bc, ic)
                # evictions: bc0/bc1 on the (by now idle) DVE, bc2/bc3
                # on ACT, so the first output half fires ~1us earlier
                # and its data drains before the second half's, and
                # each out DMA waits a single engine's sem.
                osl = O[:, bc * OUT_DIM:(bc + 1) * OUT_DIM]
                if bc < 2:
                    dve_order(nc.vector.tensor_scalar(
                        osl, psums[bc][:], 0.0, None, ALU.add))
                else:
                    nc.scalar.activation(osl, psums[bc][:], ACTF.Copy)
                if bc in (1, NBC - 1):
                    # output drains in two halves on the scalar queue.
                    # Same queue -> same rings -> in-order completion,
                    # so the final drain's single wait (the second
                    # half's sem) covers both.
                    b0, nb = (0, 2) if bc == 1 else (2, 2)
                    od = nc.scalar.dma_start(
                        out=out[b0 * P:(b0 + nb) * P, :]
                            .rearrange("(c p) j -> p c j", p=P),
                        in_=O[:, b0 * OUT_DIM:(b0 + nb) * OUT_DIM]
                            .rearrange("p (c j) -> p c j", c=nb))
                    out_dmas.append(od)

    _strip_waits(nc, out_dmas)
    return nc


def _strip_waits(nc, out_dmas):
    """Walrus allows one sync wait per instruction.  Strip the provably
    redundant waits:
     - same-engine waits on non-DMA instructions (engines are in-order
       FIFOs; DMA triggers' data movement is async, so theirs stay),
     - waits dominated by an earlier same-engine wait on the same
       monotonic semaphore with >= target value,
     - DMASW same-queue WAR waits on DMA copies,
     - the final drain keeps only the last out-DMA's update sems.
    """
    import re
    eng2sem = {"EngineType.DVE": "DVE_",
               "EngineType.Activation": "Activation_",
               "EngineType.Pool": "Pool_",
               "EngineType.PE": "PE_",
               "EngineType.SP": "SP_"}
    # monotonic data-dep sems only: barrier sems reset/decrement, so
    # value-domination logic must never touch them
    _mono = re.compile(r"^(DVE|Activation|Pool|PE|SP)_\d+$"
                       r"|^DMA(HW|SW)\d+_\d+$")

    def _wait_val(w):
        return w.wait_value if w.wait_value is not None else -1

    out_sems = set()
    if out_dmas:
        osi = out_dmas[-1].ins.sync_info
        for u in (osi.on_update if osi else []):
            if u.ant_name:
                out_sems.add(u.ant_name)

    bad = []
    for blk in nc.m.functions[0].blocks:
        covered = {}
        for inst in blk.instructions:
            si = inst.sync_info
            eng = str(inst.engine)
            if si is None or not si.on_wait:
                continue
            iname = type(inst).__name__
            pref = eng2sem.get(eng)
            if iname == "InstDMACopy":
                keep = list(si.on_wait)
                nq = [w for w in keep
                      if not (w.ant_name or "").startswith("DMASW")]
                if nq:
                    keep = nq
            else:
                keep = [w for w in si.on_wait
                        if pref is None
                        or not (w.ant_name or "").startswith(pref)]
            if iname == "InstDrain" and len(keep) > 1:
                sel = [w for w in keep if (w.ant_name or "") in out_sems]
                if sel:
                    keep = sel
            keep = [w for w in keep
                    if not (w.ant_name and _mono.match(w.ant_name))
                    or covered.get((eng, w.ant_name), -1) < _wait_val(w)]
            if len(keep) > 1:
                by_sem = {}
                for w in keep:
                    k = w.ant_name
                    if k not in by_sem or _wait_val(w) > _wait_val(by_sem[k]):
                        by_sem[k] = w
                keep = list(by_sem.values())
            for w in keep:
                if w.ant_name and _mono.match(w.ant_name):
                    k = (eng, w.ant_name)
                    if covered.get(k, -1) < _wait_val(w):
                        covered[k] = _wait_val(w)
            if len(keep) != len(si.on_wait):
                si.on_wait = keep
            if len(keep) > 1 and iname not in ("InstDrain",):
                bad.append((inst.name, iname,
                            [(w.ant_name, _wait_val(w)) for w in keep]))
    assert not bad, f"many-wait instructions remain: {bad}"


def _prep_weights(spline_coeff, spline_scaling):
    # C'[m,i,j] = (1/(6h^3)) * sum_g w[m-g] * coeff[i,j,g] * scaling[i,j]
    h = 2.0 / GRID_SIZE
    c = (spline_coeff.astype(np.float64)
         * spline_scaling.astype(np.float64)[:, :, None])  # [i, j, g]
    cp = np.zeros((NM, IN_DIM, OUT_DIM), np.float64)
    for m in range(NM):
        for g in range(max(0, m - 4), m + 1):
            cp[m] += _W_BINOM[m - g] * c[:, :, g]
    cp *= 1.0 / (6.0 * h ** 3)
    return np.ascontiguousarray(
        cp.reshape(NM * IN_DIM, OUT_DIM).astype(np.float32))


def _np_mm_dtype(mm_dtype_name):
    if mm_dtype_name == "float32":
        return np.float32
    if mm_dtype_name == "float16":
        return np.float16
    if mm_dtype_name == "bfloat16":
        import ml_dtypes
        return ml_dtypes.bfloat16
    raise ValueError(mm_dtype_name)


def _run(inputs, trace=False, mm_dtype_name="float16"):
    from concourse.bass_utils import run_bass_kernel_spmd

    key = mm_dtype_name
    if key not in _cached:
        _cached[key] = _build_nc(mm_dtype_name)
    nc = _cached[key]

    x = np.asarray(inputs["x"], np.float32)
    cw = _prep_weights(np.asarray(inputs["spline_coeff"]),
                       np.asarray(inputs["spline_scaling"]))
    cw = np.ascontiguousarray(cw.astype(_np_mm_dtype(mm_dtype_name)))
    in_maps = []
    for c in range(N_CORES):
        xc = np.ascontiguousarray(
            x[c * BC:(c + 1) * BC, :].T.astype(_np_mm_dtype(mm_dtype_name)))
        in_maps.append({"xt": xc, "cw": cw})
    res = run_bass_kernel_spmd(nc, in_maps, list(range(N_CORES)),
                               trace=trace)
    outp = np.concatenate([res.results[c]["out"] for c in range(N_CORES)],
                          axis=0).astype(np.float32)
    return outp, res


def kernel(**inputs):
    outp, _ = _run(inputs, trace=False)
    return outp
